# revision 12
# baseline (speedup 1.0000x reference)
"""Trainium2 Bass kernel for nn_LocalAttentionParallel.

Reference computation (B=4, T=4096, D=768, span=256):
    q/k/v = Linear(gelu(Linear(x)))   (three 768->768->768 MLPs, exact gelu)
    scores = (q @ k^T) / sqrt(D*span), banded causal mask (0 <= i-j < span), NO softmax
    y = scores @ v ; out = layernorm(y) * ln_w + ln_b

Sharding: 8 cores = batch(4) x sequence-halves(2). Each core processes 2048
own rows plus a 256-row left halo (zeros for the first half; handled by
per-core boundary masks). All sharding/layout prep happens on the host; the
device kernel is SPMD-uniform.

Device layouts (per core):
    xT   [128, 6, 2304]  xT[p,c,r] = x_local[r, c*128+p]      (compute dtype)
    W    [128, 6, 768]   W[p,c,o]  = w[c*128+p, o]            (compute dtype)
    scoresT[j,i] computed per k-tile for a whole query block; triangular band
    masks (scaled by 1/SCALE) multiply scores straight out of PSUM.
"""

import os
import numpy as np

import concourse.bass as bass
import concourse.tile as tile
import concourse.mybir as mybir
from concourse import bacc
from concourse.bass_utils import run_bass_kernel_spmd

AF = mybir.ActivationFunctionType
ALU = mybir.AluOpType

# problem constants
B, T, D = 4, 4096, 768
SPAN = 256
LN_EPS = 1e-5
SCALE = float(np.sqrt(D * SPAN))

P = 128
NCH = D // P          # 6 contraction chunks
N_CORES = 8
T_OWN = T // 2        # rows owned per core (2048)
T_LOC = T_OWN + SPAN  # rows incl. halo (2304)
TQ = T_OWN // P       # 16 query tiles
TK = T_LOC // P       # 18 k/v tiles
RING = 3

# compute dtype: "f16" | "f32r" | "bf16" | "f32"
CDT_NAME = os.environ.get("TRN_KERNEL_CDT", "f16")
_DT = {
    "f16": (mybir.dt.float16, np.float16),
    "bf16": (mybir.dt.bfloat16, None),  # ml_dtypes.bfloat16 resolved lazily
    "f32r": (mybir.dt.float32r, np.float32),
    "f32": (mybir.dt.float32, np.float32),
}
TPB = int(os.environ.get("TRN_KERNEL_TPB", "4" if CDT_NAME in ("f16", "bf16") else "2"))
NB = TQ // TPB
# (start_tile, n_tiles) per block; tapered tail so the final LN chains are
# short after the last matmul
if TPB == 4:
    BLOCKS = [(0, 4), (4, 4), (8, 4), (12, 2), (14, 2)]
else:
    BLOCKS = [(i * TPB, TPB) for i in range(NB)]
F32 = mybir.dt.float32


def _np_cdt():
    if CDT_NAME == "bf16":
        import ml_dtypes
        return ml_dtypes.bfloat16
    return _DT[CDT_NAME][1]


def build_module(apply_ln: bool):
    cdt = _DT[CDT_NAME][0]
    nc = bacc.Bacc("TRN2", target_bir_lowering=False, debug=False,
                   num_devices=N_CORES)

    xT = nc.dram_tensor("xT", [P, NCH, T_LOC], cdt, kind="ExternalInput")
    wd = {}
    for nm in ("qw1", "qw2", "kw1", "kw2", "vw1", "vw2"):
        wd[nm] = nc.dram_tensor(nm, [P, NCH, D], cdt, kind="ExternalInput")
    b1d = nc.dram_tensor("b1", [P, 3, NCH], F32, kind="ExternalInput")
    b2d = nc.dram_tensor("b2", [P, 2, NCH], F32, kind="ExternalInput")
    vb2d = nc.dram_tensor("vb2bc", [P, D], F32, kind="ExternalInput")
    maskd = nc.dram_tensor("masks", [P, 2, 3 * P], F32, kind="ExternalInput")
    if apply_ln:
        lnwd = nc.dram_tensor("lnw", [P, D], F32, kind="ExternalInput")
        lnbd = nc.dram_tensor("lnb", [P, D], F32, kind="ExternalInput")
    yd = nc.dram_tensor("y", [P, TQ, D], F32, kind="ExternalOutput")

    BC = TPB * P  # columns (rows of x) per block

    with tile.TileContext(nc) as tc:
        with (
            tc.tile_pool(name="const", bufs=1) as cp,
            tc.tile_pool(name="work", bufs=1) as wp,
            tc.tile_pool(name="psum", bufs=1, space="PSUM") as pp,
        ):
            # ---- constants ----
            # small tensors ride SWDGE (parallel path); big weights are
            # ordered by first use and split across the two HWDGE rings
            # (sync=SP, scalar=ACT) so the prologue can start ASAP.
            wsb = {}
            for nm in wd:
                wsb[nm] = cp.tile([P, NCH, D], cdt, tag=f"w_{nm}",
                                  name=f"w_{nm}")
            # prologue-critical loads first on the SP ring; the rest by
            # first-use order on the ACT ring. SWDGE is avoided (slow start).
            xp = wp.tile([P, NCH, BC], cdt, tag="xT", bufs=2, name="xTb")
            nc.sync.dma_start(out=xp[:, :, :SPAN], in_=xT[:, :, 0:SPAN])
            b1t = cp.tile([P, 3, NCH], F32, tag="b1t", name="b1t")
            nc.sync.dma_start(out=b1t, in_=b1d[:])
            nc.sync.dma_start(out=wsb["kw1"], in_=wd["kw1"][:])
            b2t = cp.tile([P, 2, NCH], F32, tag="b2t", name="b2t")
            nc.sync.dma_start(out=b2t, in_=b2d[:])
            nc.sync.dma_start(out=wsb["kw2"], in_=wd["kw2"][:])
            nc.sync.dma_start(out=wsb["vw1"], in_=wd["vw1"][:])
            nc.sync.dma_start(out=wsb["qw1"], in_=wd["qw1"][:])
            vb2t = cp.tile([P, D], F32, tag="vb2t", name="vb2t")
            nc.sync.dma_start(out=vb2t, in_=vb2d[:])
            nc.sync.dma_start(out=wsb["vw2"], in_=wd["vw2"][:])
            nc.sync.dma_start(out=wsb["qw2"], in_=wd["qw2"][:])
            maskt = cp.tile([P, 2, 3 * P], F32, tag="maskt", name="maskt")
            nc.gpsimd.dma_start(out=maskt, in_=maskd[:])
            epst = cp.tile([P, 1], F32, tag="epst", name="epst")
            nc.vector.memset(epst, LN_EPS)
            if apply_ln:
                lnwt = cp.tile([P, D], F32, tag="lnwt", name="lnwt")
                nc.scalar.dma_start(out=lnwt, in_=lnwd[:])
                lnbt = cp.tile([P, D], F32, tag="lnbt", name="lnbt")
                nc.scalar.dma_start(out=lnbt, in_=lnbd[:])

            def ps512(ncols):
                t = pp.tile([P, 512], F32, tag="psA", bufs=4, name="psA")
                return t[:, :ncols]

            def ps768():
                return pp.tile([P, D], F32, tag="psB", bufs=2, name="psB")

            def stage1(xblk, w1, bj, ncols):
                """hT = gelu(w1.T @ xT + b1) -> [P, NCH, ncols] (cdt)."""
                h = wp.tile([P, NCH, BC], cdt, tag="hT", bufs=2, name="hT")
                for m in range(NCH):
                    ps = ps512(ncols)
                    for c in range(NCH):
                        nc.tensor.matmul(
                            ps, wsb[w1][:, c, m * P:(m + 1) * P],
                            xblk[:, c, :ncols],
                            start=(c == 0), stop=(c == NCH - 1))
                    nc.scalar.activation(h[:, m, :ncols], ps, AF.Gelu,
                                         bias=b1t[:, bj, m:m + 1], scale=1.0)
                return h

            def stage2_T(h, w2, bj, writes, ncols):
                """out chunk [o, ncols] = w2.T @ h + b2; scatter to writes."""
                for o in range(NCH):
                    ps = ps512(ncols)
                    for m in range(NCH):
                        nc.tensor.matmul(
                            ps, wsb[w2][:, m, o * P:(o + 1) * P],
                            h[:, m, :ncols],
                            start=(m == 0), stop=(m == NCH - 1))
                    for dst_tile, dst_c0, src_c0, w in writes:
                        nc.vector.tensor_scalar_add(
                            dst_tile[:, o, dst_c0:dst_c0 + w],
                            ps[:, src_c0:src_c0 + w],
                            b2t[:, bj, o:o + 1])

            def stage2_v(h, vslot, t0, ntiles):
                """v row-tiles [rows, D] = h.T @ vw2 + vb2."""
                for t in range(ntiles):
                    ps = ps768()
                    for c0, cw in ((0, 512), (512, 256)):
                        for m in range(NCH):
                            nc.tensor.matmul(
                                ps[:, c0:c0 + cw],
                                h[:, m, t * P:(t + 1) * P],
                                wsb["vw2"][:, m, c0:c0 + cw],
                                start=(m == 0), stop=(m == NCH - 1))
                    nc.vector.tensor_add(vslot[:, t0 + t, :], ps, vb2t)

            def new_kv_slot():
                k = wp.tile([P, NCH, BC], cdt, tag="kring", bufs=RING, name="kring")
                v = wp.tile([P, TPB, D], cdt, tag="vring", bufs=RING, name="vring")
                return k, v

            # k/v tiles tracked by absolute tile index: kt -> (tile, pos)
            ktile = {}
            vtile = {}

            # ---- prologue: k/v for halo tiles 0..1 ----
            kp, vp = new_kv_slot()
            ktile[0] = (kp, 0)
            ktile[1] = (kp, 1)
            vtile[0] = (vp, 0)
            vtile[1] = (vp, 1)
            hk = stage1(xp, "kw1", 1, SPAN)
            stage2_T(hk, "kw2", 1, [(kp, 0, 0, SPAN)], SPAN)
            hv = stage1(xp, "vw1", 2, SPAN)
            stage2_v(hv, vp, 0, 2)

            # ---- main blocks over own query tiles ----
            for b, (s0, n) in enumerate(BLOCKS):
                ncols = n * P
                c0 = SPAN + s0 * P  # first local row of this block
                xb = wp.tile([P, NCH, BC], cdt, tag="xT", bufs=2, name="xTb")
                nc.scalar.dma_start(out=xb[:, :, :ncols],
                                    in_=xT[:, :, c0:c0 + ncols])

                kb, vb = new_kv_slot()
                for t in range(n):
                    ktile[s0 + 2 + t] = (kb, t)
                    vtile[s0 + 2 + t] = (vb, t)

                # q for own rows (same x tiles)
                qT = wp.tile([P, NCH, BC], cdt, tag="qT", bufs=2, name="qT")
                hq = stage1(xb, "qw1", 0, ncols)
                stage2_T(hq, "qw2", 0, [(qT, 0, 0, ncols)], ncols)
                # k, v for the same x tiles (k/v tiles s0+2 .. s0+n+1)
                hk = stage1(xb, "kw1", 1, ncols)
                stage2_T(hk, "kw2", 1, [(kb, 0, 0, ncols)], ncols)
                hv = stage1(xb, "vw1", 2, ncols)
                stage2_v(hv, vb, 0, n)

                # ---- attention: scoresT per k-tile over the whole q block ----
                scs = []
                for dlt in range(n + 2):
                    p_lo = max(0, dlt - 2)
                    p_hi = min(n - 1, dlt)
                    W = (p_hi - p_lo + 1) * P
                    qoff = p_lo * P
                    moff = (3 * P - W) if dlt <= n - 1 else 0
                    msel = 1 if (b == 0 and dlt < 2) else 0
                    kts, kpos = ktile[s0 + dlt]
                    ps = ps512(W)
                    for c in range(NCH):
                        nc.tensor.matmul(
                            ps, kts[:, c, kpos * P:(kpos + 1) * P],
                            qT[:, c, qoff:qoff + W],
                            start=(c == 0), stop=(c == NCH - 1))
                    sc = wp.tile([P, 3 * P], cdt, tag="sc", bufs=6, name="sc")
                    nc.vector.tensor_mul(sc[:, :W], ps,
                                         maskt[:, msel, moff:moff + W])
                    scs.append((sc, qoff))

                yb = wp.tile([P, TPB, D], F32, tag="yb", bufs=2, name="yb")
                for p in range(n):
                    psy = ps768()
                    for j, dlt in enumerate((p, p + 1, p + 2)):
                        sc, qoff = scs[dlt]
                        soff = p * P - qoff
                        vts, vpos = vtile[s0 + dlt]
                        for c0v, cw in ((0, 512), (512, 256)):
                            nc.tensor.matmul(
                                psy[:, c0v:c0v + cw],
                                sc[:, soff:soff + P],
                                vts[:, vpos, c0v:c0v + cw],
                                start=(j == 0), stop=(j == 2))
                    # layernorm over D (in late blocks, stage through SBUF so
                    # the PSUM bank frees before the LN chain ends)
                    if b >= len(BLOCKS) - 2:
                        ycp = wp.tile([P, D], F32, tag="ycp", bufs=2,
                                      name="ycp")
                        nc.vector.tensor_copy(ycp, psy)
                        psy = ycp
                    st = wp.tile([P, 2, 6], F32, tag="st", bufs=4, name="st")
                    for s in range(2):
                        nc.vector.bn_stats(st[:, s, :],
                                           psy[:, s * 384:(s + 1) * 384])
                    mv = wp.tile([P, 2], F32, tag="mv", bufs=4, name="mv")
                    nc.vector.bn_aggr(mv, st)
                    sd = wp.tile([P, 1], F32, tag="sd", bufs=4, name="sd")
                    nc.scalar.activation(sd, mv[:, 1:2], AF.Sqrt,
                                         bias=epst, scale=1.0)
                    rs = wp.tile([P, 1], F32, tag="rs", bufs=4, name="rs")
                    nc.vector.reciprocal(rs, sd)
                    nc.vector.tensor_scalar(
                        out=yb[:, p, :], in0=psy,
                        scalar1=mv[:, 0:1], scalar2=rs,
                        op0=ALU.subtract, op1=ALU.mult)
                    if apply_ln:
                        nc.vector.tensor_mul(yb[:, p, :], yb[:, p, :], lnwt)
                        nc.vector.tensor_add(yb[:, p, :], yb[:, p, :], lnbt)
                    nc.scalar.dma_start(out=yd[:, s0 + p, :],
                                        in_=yb[:, p, :])


    nc.compile()
    return nc


def _make_masks(h):
    jj, ii = np.mgrid[0:P, 0:P]
    diag = (ii >= jj).astype(np.float32)
    full = np.ones((P, P), np.float32)
    strict = (ii < jj).astype(np.float32)
    gen = np.concatenate([diag, full, strict], axis=1) / SCALE
    if h == 0:
        z = np.zeros((P, P), np.float32)
        blk0 = np.concatenate([diag / SCALE, z, z], axis=1)
    else:
        blk0 = gen
    return np.stack([gen, blk0], axis=1)  # [P, 2, 384]


def kernel(**inputs):
    x = np.asarray(inputs["x"], np.float32)
    npdt = _np_cdt()

    ln_w = np.asarray(inputs["ln_w"], np.float32)
    ln_b = np.asarray(inputs["ln_b"], np.float32)
    apply_ln = not (np.all(ln_w == 1.0) and np.all(ln_b == 0.0))

    nc = build_module(apply_ln)

    def warr(nm):
        w = np.asarray(inputs[nm], np.float32)
        return np.ascontiguousarray(
            w.reshape(NCH, P, D).transpose(1, 0, 2)).astype(npdt)

    wmats = {nm: warr(nm) for nm in ("qw1", "qw2", "kw1", "kw2", "vw1", "vw2")}
    b1 = np.ascontiguousarray(
        np.stack([inputs["qb1"], inputs["kb1"], inputs["vb1"]])
        .astype(np.float32).reshape(3, NCH, P).transpose(2, 0, 1))
    b2 = np.ascontiguousarray(
        np.stack([inputs["qb2"], inputs["kb2"]])
        .astype(np.float32).reshape(2, NCH, P).transpose(2, 0, 1))
    vb2bc = np.ascontiguousarray(
        np.broadcast_to(np.asarray(inputs["vb2"], np.float32), (P, D)))

    in_maps = []
    for c in range(N_CORES):
        bi, h = c // 2, c % 2
        xl = np.zeros((T_LOC, D), np.float32)
        lo = h * T_OWN - SPAN
        if h == 0:
            xl[SPAN:] = x[bi, 0:T_OWN]
        else:
            xl[:] = x[bi, lo:lo + T_LOC]
        xTn = np.ascontiguousarray(
            xl.T.reshape(NCH, P, T_LOC).transpose(1, 0, 2)).astype(npdt)
        m = {
            "xT": xTn, "b1": b1, "b2": b2, "vb2bc": vb2bc,
            "masks": np.ascontiguousarray(_make_masks(h)),
        }
        m.update(wmats)
        if apply_ln:
            m["lnw"] = np.ascontiguousarray(np.broadcast_to(ln_w, (P, D)))
            m["lnb"] = np.ascontiguousarray(np.broadcast_to(ln_b, (P, D)))
        in_maps.append(m)

    trace = os.environ.get("TRN_KERNEL_TRACE", "0") == "1"
    res = run_bass_kernel_spmd(nc, in_maps, core_ids=list(range(N_CORES)),
                               trace=trace)
    if trace and res.exec_time_ns is not None:
        print(f"HW exec time: {res.exec_time_ns} ns")
        print(f"mean exec time: {res.mean_exec_time_ns} ns")
        if res.instructions_and_trace is not None:
            print(f"trace: {res.instructions_and_trace[1]}")

    out = np.empty((B, T, D), np.float32)
    for c in range(N_CORES):
        bi, h = c // 2, c % 2
        yc = res.results[c]["y"]  # [P, TQ, D]
        out[bi, h * T_OWN:(h + 1) * T_OWN] = (
            yc.transpose(1, 0, 2).reshape(T_OWN, D))
    return out
